# revision 5
# baseline (speedup 1.0000x reference)
"""Entmax-1.5 (alpha-entmax via bisection) Trainium2 kernel, v8.

Problem: p = entmax_bisect(where(mask, scores, -1e9), alpha=1.5) over the
last dim of a [16384, 4096] f32 tensor, data-parallel over 8 NeuronCores
(2048 rows per core).

Math: for alpha=1.5, p_i = relu(z_i - tau)^2 / f(tau) with
f(sigma) = sum relu(z - sigma)^2 and f(tau) = 4 at the root.  tau is
located with CHEAP probes on the 512-wide max-of-8 tree level (fc), then
one full-width f-eval; normalization is exact on the host:

  tree:   fa/fb/fc pairwise-max levels; fc keeps every active lane's
          group max, so sum relu(fc-s)^2 ~= f(s) near the root.
  sigma0 = C0 + C1*m + C2*mean(fc)                    (regression)
  round1: fc-probe (f,g) -> frozen-set quadratic in delta form:
          d = (g - sqrt(g^2 - n(f-4)))/n, n = NS*g^2/f.
  round2: fc-probe (f,g) -> Newton d = (f-4)/(2g).
  full:   FF = sum relu(z-sigma2)^2 (exact);
          tau = sigma2 + (FF-4)/(2*(g2 - n1*d2))      (model-g Newton).
  final:  v = relu(z-tau); p16 = v^2 (fp16); host divides by the row
          sum (ensure_sum_one exact) and casts to f32.

Probe (f,g) extraction per tile: either DVE tensor_scalar max+accumulate
(g, with S*sigma correction) + ScalarE Square(bias) accumulate (f), or
both on ScalarE (Relu w/ bias+accumulate, then Square accumulate) --
PK tiles per group use the ScalarE form to balance the engines.
Validated vs the jax reference on all 16384 rows: norm_rel ~1.9e-3.
"""

import numpy as np

P = 128          # SBUF partitions
S = 4096         # row length
SC = 512         # fc width (max-of-8)
MUW = 256        # subsample width for the regression mean
B_FULL = 16384   # total rows
N_CORES = 8
BP = B_FULL // N_CORES   # rows per core
NT = BP // P             # 16 tiles of 128 rows per core
G = 8                    # tiles per stat group
NG = NT // G             # stat groups per core

NEG = -30.0              # mask stand-in for -inf
CLAMP_HI = 0.0312        # tau <= m - 2*sqrt(1/S)
REG = (-0.0227, 0.3391, 0.9736)   # sigma0 = c0 + c1*m + c2*mean(fc)
NS = 1.1                 # n_hat scale in the quadratic
FLOOR = 1e-9

PK = (3, 3)      # per-group fc-probe legs on ScalarE (per round)
ACT_K = (0, 0)   # per-group full-eval relu legs on ScalarE
FDV = (3, 4)     # per-group final squares on DVE (scalar_tensor_tensor)

_CACHE = {}


def _build_program():
    import concourse.bacc as bacc
    import concourse.tile as tile
    import concourse.mybir as mybir
    from contextlib import ExitStack

    f32 = mybir.dt.float32
    f16 = mybir.dt.float16
    Alu = mybir.AluOpType
    Act = mybir.ActivationFunctionType
    X = mybir.AxisListType.X

    nc = bacc.Bacc(
        "TRN2",
        target_bir_lowering=False,
        debug=False,
        enable_asserts=False,
        num_devices=N_CORES,
    )
    z_d = nc.dram_tensor("z16", [BP, S], f16, kind="ExternalInput").ap()
    out_d = nc.dram_tensor("out", [BP, S], f16, kind="ExternalOutput").ap()

    with tile.TileContext(nc) as tc, ExitStack() as ctx:
        z_pool = ctx.enter_context(tc.tile_pool(name="z", bufs=NT + 1))
        fc_pool = ctx.enter_context(tc.tile_pool(name="fc", bufs=G + 2))
        f_pool = ctx.enter_context(tc.tile_pool(name="fab", bufs=2))
        r_pool = ctx.enter_context(tc.tile_pool(name="rfc", bufs=4))
        w_pool = ctx.enter_context(tc.tile_pool(name="w", bufs=3))
        p_pool = ctx.enter_context(tc.tile_pool(name="p", bufs=2))
        s_pool = ctx.enter_context(tc.tile_pool(name="st", bufs=2))
        c_pool = ctx.enter_context(tc.tile_pool(name="cst", bufs=1))

        def st(name, gi, dt=f32):
            return s_pool.tile([P, G], dt, tag=name, name=f"{name}_{gi}")

        # per-column correction vector: SC for DVE-leg probe columns
        # (accumulate max(fc,sig) -> g = acc - SC*sig), 0 for ScalarE legs
        # (accumulate relu directly).
        svg, psets = [], []
        for gi in range(NG):
            pk = PK[gi]
            pset = {(i * G) // pk + G // (2 * pk) for i in range(pk)} if pk else set()
            sv = c_pool.tile([P, G], f32, tag=f"sv_{gi}", name=f"sv_{gi}")
            for t in range(G):
                nc.vector.memset(sv[:, t : t + 1], 0.0 if t in pset else float(SC))
            svg.append(sv)
            psets.append(pset)

        zs = [None] * NT
        fcs = [None] * NT
        grp = []
        for gi in range(NG):
            g = {}
            grp.append(g)
            for nm in ("M", "CH", "S0", "MU", "SIG", "NSG", "GMR", "FQ",
                       "GA", "GQ", "QN", "D2", "FF", "TAU", "t1", "t2",
                       "t3", "t4", "t5", "t6"):
                g[nm] = st(nm, gi)
            g["SH"] = st("SH", gi, f16)

        # ---- stage 1: load + max tree + mean(fc) ---------------------
        def stage1(gi, lo=0, hi=G):
            g = grp[gi]
            for t in range(lo, hi):
                ti = gi * G + t
                row0 = ti * P
                z_t = z_pool.tile([P, S], f16, tag="z", name=f"z_{ti}")
                nc.sync.dma_start(z_t[:], z_d[row0 : row0 + P, :])
                zs[ti] = z_t
                fa = f_pool.tile([P, S // 2], f16, tag="fa", name=f"fa_{ti}")
                nc.vector.tensor_tensor(
                    out=fa[:], in0=z_t[:, 0 : S // 2], in1=z_t[:, S // 2 : S],
                    op=Alu.max,
                )
                fb = f_pool.tile([P, S // 4], f16, tag="fb", name=f"fb_{ti}")
                nc.vector.tensor_tensor(
                    out=fb[:], in0=fa[:, 0 : S // 4], in1=fa[:, S // 4 : S // 2],
                    op=Alu.max,
                )
                fc_t = fc_pool.tile([P, SC], f16, tag="fc", name=f"fc_{ti}")
                nc.vector.tensor_tensor(
                    out=fc_t[:], in0=fb[:, 0 : SC], in1=fb[:, SC : 2 * SC],
                    op=Alu.max,
                )
                fcs[ti] = fc_t
                nc.vector.reduce_max(g["M"][:, t : t + 1], fc_t[:], axis=X)
                j = r_pool.tile([P, MUW], f16, tag="ju", name=f"ju_{ti}")
                nc.vector.tensor_scalar(
                    out=j[:], in0=fc_t[:, 0:MUW], scalar1=0.0, scalar2=None,
                    op0=Alu.add, op1=Alu.add, accum_out=g["MU"][:, t : t + 1],
                )

        # ---- stage 2: regression sigma0 ------------------------------
        def stage2(gi):
            g = grp[gi]
            nc.vector.tensor_scalar(
                out=g["CH"][:], in0=g["M"][:], scalar1=-CLAMP_HI, scalar2=None,
                op0=Alu.add,
            )
            nc.vector.tensor_scalar(
                out=g["S0"][:], in0=g["M"][:], scalar1=-2.0, scalar2=None,
                op0=Alu.add,
            )
            nc.vector.tensor_scalar(
                out=g["t1"][:], in0=g["M"][:], scalar1=REG[1], scalar2=REG[0],
                op0=Alu.mult, op1=Alu.add,
            )
            nc.vector.scalar_tensor_tensor(
                out=g["SIG"][:], in0=g["MU"][:], scalar=REG[2] / MUW,
                in1=g["t1"][:], op0=Alu.mult, op1=Alu.add,
            )
            nc.vector.tensor_tensor(out=g["SIG"][:], in0=g["SIG"][:], in1=g["CH"][:], op=Alu.min)
            nc.vector.tensor_tensor(out=g["SIG"][:], in0=g["SIG"][:], in1=g["S0"][:], op=Alu.max)
            nc.vector.tensor_copy(g["SH"][:], g["SIG"][:])
            nc.vector.tensor_copy(g["SIG"][:], g["SH"][:])
            nc.vector.tensor_scalar(
                out=g["NSG"][:], in0=g["SIG"][:], scalar1=-1.0, scalar2=None,
                op0=Alu.mult,
            )

        # ---- fc-probe at SIG: fills GA (raw) and FQ ------------------
        def fc_probe(gi, rnd):
            g = grp[gi]
            for t in range(G):
                ti = gi * G + t
                rfc = r_pool.tile([P, SC], f16, tag="rfc", name=f"r{rnd}_{ti}")
                if t in psets[gi]:
                    nc.scalar.activation(
                        rfc[:], fcs[ti][:], Act.Relu,
                        bias=g["NSG"][:, t : t + 1],
                        accum_out=g["GA"][:, t : t + 1],
                    )
                    nc.scalar.activation(
                        rfc[:], rfc[:], Act.Square,
                        accum_out=g["FQ"][:, t : t + 1],
                    )
                else:
                    nc.vector.tensor_scalar(
                        out=rfc[:], in0=fcs[ti][:],
                        scalar1=g["SIG"][:, t : t + 1], scalar2=None,
                        op0=Alu.max, op1=Alu.add,
                        accum_out=g["GA"][:, t : t + 1],
                    )
                    nc.scalar.activation(
                        rfc[:], rfc[:], Act.Square,
                        bias=g["SIG"][:, t : t + 1], scale=-1.0,
                        accum_out=g["FQ"][:, t : t + 1],
                    )

        def g_corr(gi):
            g = grp[gi]
            nc.vector.tensor_tensor(out=g["t1"][:], in0=svg[gi][:], in1=g["SIG"][:], op=Alu.mult)
            nc.vector.tensor_tensor(out=g["GQ"][:], in0=g["GA"][:], in1=g["t1"][:], op=Alu.subtract)
            nc.vector.tensor_scalar(
                out=g["GQ"][:], in0=g["GQ"][:], scalar1=FLOOR, scalar2=None, op0=Alu.max,
            )

        # ---- round-1 quadratic (delta form) --------------------------
        def quad1(gi):
            g = grp[gi]
            g_corr(gi)
            nc.vector.tensor_tensor(out=g["t6"][:], in0=g["GQ"][:], in1=g["GQ"][:], op=Alu.mult)
            nc.vector.tensor_scalar(
                out=g["t2"][:], in0=g["FQ"][:], scalar1=FLOOR, scalar2=None, op0=Alu.max,
            )
            nc.vector.reciprocal(g["t4"][:], g["t2"][:])
            nc.vector.tensor_tensor(out=g["t3"][:], in0=g["t6"][:], in1=g["t4"][:], op=Alu.mult)
            nc.vector.tensor_scalar(
                out=g["t3"][:], in0=g["t3"][:], scalar1=NS, scalar2=1.0,
                op0=Alu.mult, op1=Alu.max,
            )
            nc.vector.tensor_scalar(
                out=g["t5"][:], in0=g["FQ"][:], scalar1=-4.0, scalar2=None, op0=Alu.add,
            )
            nc.vector.tensor_tensor(out=g["t5"][:], in0=g["t3"][:], in1=g["t5"][:], op=Alu.mult)
            nc.vector.tensor_tensor(out=g["t5"][:], in0=g["t6"][:], in1=g["t5"][:], op=Alu.subtract)
            nc.vector.tensor_scalar(
                out=g["t5"][:], in0=g["t5"][:], scalar1=0.0, scalar2=None, op0=Alu.max,
            )
            nc.scalar.activation(g["t5"][:], g["t5"][:], Act.Sqrt)
            nc.vector.tensor_tensor(out=g["t5"][:], in0=g["GQ"][:], in1=g["t5"][:], op=Alu.subtract)
            nc.vector.reciprocal(g["t4"][:], g["t3"][:])
            nc.vector.tensor_tensor(out=g["t5"][:], in0=g["t5"][:], in1=g["t4"][:], op=Alu.mult)
            nc.vector.tensor_tensor(out=g["SIG"][:], in0=g["SIG"][:], in1=g["t5"][:], op=Alu.add)
            nc.vector.tensor_tensor(out=g["SIG"][:], in0=g["SIG"][:], in1=g["CH"][:], op=Alu.min)
            nc.vector.tensor_tensor(out=g["SIG"][:], in0=g["SIG"][:], in1=g["S0"][:], op=Alu.max)
            nc.vector.tensor_copy(g["SH"][:], g["SIG"][:])
            nc.vector.tensor_copy(g["SIG"][:], g["SH"][:])
            nc.vector.tensor_copy(g["QN"][:], g["t3"][:])
            nc.vector.tensor_scalar(
                out=g["NSG"][:], in0=g["SIG"][:], scalar1=-1.0, scalar2=None,
                op0=Alu.mult,
            )

        # ---- round-2 Newton ------------------------------------------
        def newton2(gi):
            g = grp[gi]
            g_corr(gi)
            nc.vector.reciprocal(g["t2"][:], g["GQ"][:])
            nc.vector.tensor_scalar(
                out=g["t5"][:], in0=g["FQ"][:], scalar1=-4.0, scalar2=None, op0=Alu.add,
            )
            nc.vector.tensor_tensor(out=g["t5"][:], in0=g["t5"][:], in1=g["t2"][:], op=Alu.mult)
            nc.vector.tensor_scalar(
                out=g["D2"][:], in0=g["t5"][:], scalar1=0.5, scalar2=None, op0=Alu.mult,
            )
            nc.vector.tensor_tensor(out=g["SIG"][:], in0=g["SIG"][:], in1=g["D2"][:], op=Alu.add)
            nc.vector.tensor_tensor(out=g["SIG"][:], in0=g["SIG"][:], in1=g["CH"][:], op=Alu.min)
            nc.vector.tensor_tensor(out=g["SIG"][:], in0=g["SIG"][:], in1=g["S0"][:], op=Alu.max)
            nc.vector.tensor_copy(g["SH"][:], g["SIG"][:])
            nc.vector.tensor_copy(g["SIG"][:], g["SH"][:])
            # gmod = max(g2 - n1*d2, FLOOR); GMR = 0.5/gmod
            nc.vector.tensor_tensor(out=g["t1"][:], in0=g["QN"][:], in1=g["D2"][:], op=Alu.mult)
            nc.vector.tensor_tensor(out=g["t1"][:], in0=g["GQ"][:], in1=g["t1"][:], op=Alu.subtract)
            nc.vector.tensor_scalar(
                out=g["t1"][:], in0=g["t1"][:], scalar1=FLOOR, scalar2=None, op0=Alu.max,
            )
            nc.vector.reciprocal(g["t1"][:], g["t1"][:])
            nc.vector.tensor_scalar(
                out=g["GMR"][:], in0=g["t1"][:], scalar1=0.5, scalar2=None, op0=Alu.mult,
            )
            if ACT_K[gi] > 0:
                nc.vector.tensor_scalar(
                    out=g["NSG"][:], in0=g["SIG"][:], scalar1=-1.0, scalar2=None,
                    op0=Alu.mult,
                )

        # ---- stage 7: full-width f eval ------------------------------
        def stage7(gi):
            g = grp[gi]
            a = ACT_K[gi]
            actset = {(i * G) // a + G // (2 * a) for i in range(a)} if a else set()
            for t in range(G):
                ti = gi * G + t
                w_t = w_pool.tile([P, S], f16, tag="w", name=f"w_{ti}")
                if t in actset:
                    nc.scalar.activation(
                        w_t[:], zs[ti][:], Act.Relu, bias=g["NSG"][:, t : t + 1],
                    )
                else:
                    nc.vector.tensor_scalar(
                        out=w_t[:], in0=zs[ti][:],
                        scalar1=g["SIG"][:, t : t + 1], scalar2=g["SIG"][:, t : t + 1],
                        op0=Alu.max, op1=Alu.subtract,
                    )
                nc.scalar.activation(
                    w_t[:], w_t[:], Act.Square, accum_out=g["FF"][:, t : t + 1],
                )

        # ---- stage 8: Newton tau -------------------------------------
        def stage8(gi):
            g = grp[gi]
            nc.vector.tensor_scalar(
                out=g["t1"][:], in0=g["FF"][:], scalar1=-4.0, scalar2=None, op0=Alu.add,
            )
            nc.vector.tensor_tensor(out=g["t1"][:], in0=g["t1"][:], in1=g["GMR"][:], op=Alu.mult)
            nc.vector.tensor_tensor(out=g["TAU"][:], in0=g["SIG"][:], in1=g["t1"][:], op=Alu.add)
            nc.vector.tensor_tensor(out=g["TAU"][:], in0=g["TAU"][:], in1=g["CH"][:], op=Alu.min)
            nc.vector.tensor_tensor(out=g["TAU"][:], in0=g["TAU"][:], in1=g["S0"][:], op=Alu.max)
            nc.vector.tensor_copy(g["SH"][:], g["TAU"][:])
            nc.vector.tensor_copy(g["TAU"][:], g["SH"][:])

        # ---- stage 9: final pass + store -----------------------------
        def stage9(gi):
            g = grp[gi]
            d = FDV[gi]
            dveset = {(i * G) // d + G // (2 * d) for i in range(d)} if d else set()
            for t in range(G):
                ti = gi * G + t
                row0 = ti * P
                v_t = w_pool.tile([P, S], f16, tag="w", name=f"v_{ti}")
                nc.vector.tensor_scalar(
                    out=v_t[:], in0=zs[ti][:],
                    scalar1=g["TAU"][:, t : t + 1], scalar2=g["TAU"][:, t : t + 1],
                    op0=Alu.max, op1=Alu.subtract,
                )
                p_t = p_pool.tile([P, S], f16, tag="p", name=f"p_{ti}")
                if t in dveset:
                    nc.vector.scalar_tensor_tensor(
                        out=p_t[:], in0=v_t[:], scalar=1.0, in1=v_t[:],
                        op0=Alu.mult, op1=Alu.mult,
                    )
                else:
                    nc.scalar.activation(p_t[:], v_t[:], Act.Square)
                nc.sync.dma_start(out_d[row0 : row0 + P, :], p_t[:])

        # ---- software-pipelined issue order (NG=2) -------------------
        stage1(0)
        stage2(0)
        fc_probe(0, 0)
        quad1(0)
        fc_probe(0, 1)
        newton2(0)
        stage7(0)
        stage1(1, 0, 4)
        stage2_done = False
        stage8(0)
        stage9(0)
        stage1(1, 4, 8)
        stage2(1)
        fc_probe(1, 0)
        quad1(1)
        fc_probe(1, 1)
        newton2(1)
        stage7(1)
        stage8(1)
        stage9(1)

    nc.compile()
    return nc


def _get_program():
    if "nc" not in _CACHE:
        _CACHE["nc"] = _build_program()
    return _CACHE["nc"]


def _prep_z16(scores, mask_b):
    z16 = scores.astype(np.float16)
    np.copyto(z16, np.float16(NEG), where=~mask_b)
    return np.ascontiguousarray(z16)


def _kernel_numpy_fallback(scores, mask, alpha):
    """Reference-equivalent host computation (only for alpha != 1.5)."""
    f32 = np.float32
    alpha = max(float(alpha), 1.0)
    am1 = alpha - 1.0
    x = np.where(mask, scores, f32(-1e9)).astype(f32)
    Xs = (x * f32(am1)).astype(f32)
    mx = Xs.max(axis=-1, keepdims=True)
    tau_lo = mx - f32(1.0)
    tau_hi = mx - f32((1.0 / x.shape[-1]) ** am1)
    dm = tau_hi - tau_lo
    tau_m = tau_lo
    inv = f32(1.0 / am1)
    for _ in range(50):
        dm = dm / 2
        tau_m = tau_lo + dm
        p = np.clip(Xs - tau_m, 0.0, None) ** inv
        f = p.sum(axis=-1, keepdims=True) - 1.0
        tau_lo = np.where(f >= 0, tau_m, tau_lo)
    p = np.clip(Xs - tau_m, 0.0, None) ** inv
    return (p / p.sum(axis=-1, keepdims=True)).astype(f32)


def kernel(scores, mask, alpha):
    scores = np.asarray(scores, dtype=np.float32)
    mask_b = np.asarray(mask).astype(bool)
    alpha_v = float(np.asarray(alpha))

    if abs(max(alpha_v, 1.0) - 1.5) > 1e-6:
        return _kernel_numpy_fallback(scores, mask_b, alpha_v)

    z16 = _prep_z16(scores, mask_b)

    from concourse import bass_utils

    nc = _get_program()
    in_maps = [{"z16": z16[i * BP : (i + 1) * BP]} for i in range(N_CORES)]
    res = bass_utils.run_bass_kernel_spmd(nc, in_maps, core_ids=list(range(N_CORES)))
    outs = []
    for r in res.results:
        p = r["out"].astype(np.float32)
        Z = p.sum(axis=1)
        p /= np.maximum(Z, 1e-9)[:, None]
        outs.append(p)
    return np.concatenate(outs, axis=0)


# revision 9
# speedup vs baseline: 1.2143x; 1.2143x over previous
"""Entmax-1.5 (alpha-entmax via bisection) Trainium2 kernel, v10.

Problem: p = entmax_bisect(where(mask, scores, -1e9), alpha=1.5) over the
last dim of a [16384, 4096] f32 tensor, data-parallel over 8 NeuronCores
(2048 rows per core).

Math: for alpha=1.5, p_i = relu(z_i - tau)^2 / f(tau) with
f(sigma) = sum relu(z - sigma)^2 and f(tau) = 4 at the root.  tau is
located entirely on the pairwise-max TREE levels (fb = max-of-4, 1024
wide; fc = max-of-8, 512 wide), never on the full rows:

  sum relu(level - s)^2 ~= f(s) near the root: each active lane's group
  max survives, only same-group collisions are missed (rare: the active
  set is ~22 of 4096 lanes; max-of-4 collides ~4% of rows).

  sigma0 = C0 + C1*m + C2*mean(fc)                  (regression)
  2x fc-probe  -> frozen-set quadratic in delta form:
                  d = (g - sqrt(g^2 - n(f-4)))/n, n = NS*g^2/f
  1x fb-probe  -> Newton d = (f-4)/(2g)  ->  tau
  final: v = relu(z-tau); p16 = v^2 (fp16); the host divides by the row
  sum (ensure_sum_one exact) and casts to f32 -- the exact
  normalization absorbs the f-error, only tau placement matters.

Each probe is one DVE tensor_scalar (max, stores the clipped row,
accumulates sum -> g after a W*sigma correction) plus one ScalarE
Square(bias) accumulate (-> f), so the DVE-heavy front-end and the
ScalarE work overlap from the first tile.  Inputs fp16 (host folds the
mask; -30 = -inf); output fp16 halves store-side HBM traffic.
Validated vs the jax reference on all 16384 rows: norm_rel ~4.7e-3.
"""

import numpy as np

P = 128          # SBUF partitions
S = 4096         # row length
WB = 1024        # fb width (max-of-4)
WC = 512         # fc width (max-of-8)
MUW = 256        # subsample width for the regression mean
B_FULL = 16384   # total rows
N_CORES = 8
BP = B_FULL // N_CORES   # rows per core
NT = BP // P             # 16 tiles of 128 rows per core
G = 8                    # tiles per stat group
NG = NT // G             # stat groups per core

NEG = -30.0              # mask stand-in for -inf
CLAMP_HI = 0.0312        # tau <= m - 2*sqrt(1/S)
REG = (-0.0227, 0.3391, 0.9736)   # sigma0 = c0 + c1*m + c2*mean(fc)
NS = 1.1                 # n_hat scale in the quadratic
FLOOR = 1e-9

FDV = (2, 2)     # per-group final squares on DVE (tensor_tensor v*v)

_CACHE = {}


def _build_program():
    import concourse.bacc as bacc
    import concourse.tile as tile
    import concourse.mybir as mybir
    from contextlib import ExitStack

    f32 = mybir.dt.float32
    f16 = mybir.dt.float16
    Alu = mybir.AluOpType
    Act = mybir.ActivationFunctionType
    X = mybir.AxisListType.X

    nc = bacc.Bacc(
        "TRN2",
        target_bir_lowering=False,
        debug=False,
        enable_asserts=False,
        num_devices=N_CORES,
    )
    z_d = nc.dram_tensor("z16", [BP, S], f16, kind="ExternalInput").ap()
    out_d = nc.dram_tensor("out", [BP, S], f16, kind="ExternalOutput").ap()

    with tile.TileContext(nc) as tc, ExitStack() as ctx:
        z_pool = ctx.enter_context(tc.tile_pool(name="z", bufs=NT + 1))
        fb_pool = ctx.enter_context(tc.tile_pool(name="fb", bufs=G + 1))
        fc_pool = ctx.enter_context(tc.tile_pool(name="fc", bufs=G + 2))
        f_pool = ctx.enter_context(tc.tile_pool(name="fa", bufs=1))
        r_pool = ctx.enter_context(tc.tile_pool(name="rp", bufs=2))
        rb_pool = ctx.enter_context(tc.tile_pool(name="rb", bufs=2))
        w_pool = ctx.enter_context(tc.tile_pool(name="w", bufs=2))
        p_pool = ctx.enter_context(tc.tile_pool(name="p", bufs=1))
        s_pool = ctx.enter_context(tc.tile_pool(name="st", bufs=2))

        def st(name, gi, dt=f32):
            return s_pool.tile([P, G], dt, tag=name, name=f"{name}_{gi}")

        zs = [None] * NT
        fbs = [None] * NT
        fcs = [None] * NT
        grp = []
        for gi in range(NG):
            g = {}
            grp.append(g)
            for nm in ("M", "CH", "S0", "MU", "SIG", "GA", "FQ", "GQ",
                       "t1", "t2", "t3", "t4", "t5", "t6"):
                g[nm] = st(nm, gi)

        # ---- stage 1: load + max tree + mean(fc) ---------------------
        def stage1(gi, lo=0, hi=G):
            g = grp[gi]
            for t in range(lo, hi):
                ti = gi * G + t
                row0 = ti * P
                z_t = z_pool.tile([P, S], f16, tag="z", name=f"z_{ti}")
                nc.sync.dma_start(z_t[:], z_d[row0 : row0 + P, :])
                zs[ti] = z_t
                fa = f_pool.tile([P, S // 2], f16, tag="fa", name=f"fa_{ti}")
                nc.vector.tensor_tensor(
                    out=fa[:], in0=z_t[:, 0 : S // 2], in1=z_t[:, S // 2 : S],
                    op=Alu.max,
                )
                fb_t = fb_pool.tile([P, WB], f16, tag="fb", name=f"fb_{ti}")
                nc.vector.tensor_tensor(
                    out=fb_t[:], in0=fa[:, 0:WB], in1=fa[:, WB : 2 * WB],
                    op=Alu.max,
                )
                fbs[ti] = fb_t
                fc_t = fc_pool.tile([P, WC], f16, tag="fc", name=f"fc_{ti}")
                nc.vector.tensor_tensor(
                    out=fc_t[:], in0=fb_t[:, 0:WC], in1=fb_t[:, WC : 2 * WC],
                    op=Alu.max,
                )
                fcs[ti] = fc_t
                nc.vector.reduce_max(g["M"][:, t : t + 1], fc_t[:], axis=X)
                j = r_pool.tile([P, MUW], f16, tag="ju", name=f"ju_{ti}")
                nc.vector.tensor_scalar(
                    out=j[:], in0=fc_t[:, 0:MUW], scalar1=0.0, scalar2=None,
                    op0=Alu.add, op1=Alu.add, accum_out=g["MU"][:, t : t + 1],
                )

        # ---- stage 2: regression sigma0 ------------------------------
        def stage2(gi):
            g = grp[gi]
            nc.vector.tensor_scalar(
                out=g["CH"][:], in0=g["M"][:], scalar1=-CLAMP_HI, scalar2=None,
                op0=Alu.add,
            )
            nc.vector.tensor_scalar(
                out=g["S0"][:], in0=g["M"][:], scalar1=-2.0, scalar2=None,
                op0=Alu.add,
            )
            nc.vector.tensor_scalar(
                out=g["t1"][:], in0=g["M"][:], scalar1=REG[1], scalar2=REG[0],
                op0=Alu.mult, op1=Alu.add,
            )
            nc.vector.scalar_tensor_tensor(
                out=g["SIG"][:], in0=g["MU"][:], scalar=REG[2] / MUW,
                in1=g["t1"][:], op0=Alu.mult, op1=Alu.add,
            )
            nc.vector.tensor_tensor(out=g["SIG"][:], in0=g["SIG"][:], in1=g["CH"][:], op=Alu.min)
            nc.vector.tensor_tensor(out=g["SIG"][:], in0=g["SIG"][:], in1=g["S0"][:], op=Alu.max)

        # ---- probe at SIG on a tree level ----------------------------
        def probe(gi, lvls, width, pool, tag, rnd):
            g = grp[gi]
            for t in range(G):
                ti = gi * G + t
                r = pool.tile([P, width], f16, tag=tag, name=f"r{rnd}_{ti}")
                nc.vector.tensor_scalar(
                    out=r[:], in0=lvls[ti][:],
                    scalar1=g["SIG"][:, t : t + 1], scalar2=None,
                    op0=Alu.max, op1=Alu.add,
                    accum_out=g["GA"][:, t : t + 1],
                )
                nc.scalar.activation(
                    r[:], r[:], Act.Square,
                    bias=g["SIG"][:, t : t + 1], scale=-1.0,
                    accum_out=g["FQ"][:, t : t + 1],
                )

        def g_corr(gi, width):
            g = grp[gi]
            nc.vector.scalar_tensor_tensor(
                out=g["GQ"][:], in0=g["SIG"][:], scalar=-float(width),
                in1=g["GA"][:], op0=Alu.mult, op1=Alu.add,
            )

        # ---- quadratic update (delta form) ---------------------------
        def quad(gi, width):
            g = grp[gi]
            g_corr(gi, width)
            nc.vector.tensor_scalar(
                out=g["t2"][:], in0=g["FQ"][:], scalar1=FLOOR, scalar2=None, op0=Alu.max,
            )
            nc.vector.tensor_tensor(out=g["t6"][:], in0=g["GQ"][:], in1=g["GQ"][:], op=Alu.mult)
            nc.vector.reciprocal(g["t4"][:], g["t2"][:])
            nc.vector.tensor_tensor(out=g["t3"][:], in0=g["t6"][:], in1=g["t4"][:], op=Alu.mult)
            nc.vector.tensor_scalar(
                out=g["t3"][:], in0=g["t3"][:], scalar1=NS, scalar2=1.0,
                op0=Alu.mult, op1=Alu.max,
            )
            nc.vector.tensor_scalar(
                out=g["t5"][:], in0=g["FQ"][:], scalar1=-4.0, scalar2=None, op0=Alu.add,
            )
            nc.vector.tensor_tensor(out=g["t5"][:], in0=g["t3"][:], in1=g["t5"][:], op=Alu.mult)
            nc.vector.tensor_tensor(out=g["t5"][:], in0=g["t6"][:], in1=g["t5"][:], op=Alu.subtract)
            nc.vector.tensor_scalar(
                out=g["t5"][:], in0=g["t5"][:], scalar1=0.0, scalar2=None, op0=Alu.max,
            )
            nc.scalar.activation(g["t5"][:], g["t5"][:], Act.Sqrt)
            nc.vector.tensor_tensor(out=g["t5"][:], in0=g["GQ"][:], in1=g["t5"][:], op=Alu.subtract)
            nc.vector.reciprocal(g["t4"][:], g["t3"][:])
            nc.vector.tensor_tensor(out=g["t5"][:], in0=g["t5"][:], in1=g["t4"][:], op=Alu.mult)
            nc.vector.tensor_tensor(out=g["SIG"][:], in0=g["SIG"][:], in1=g["t5"][:], op=Alu.add)
            nc.vector.tensor_tensor(out=g["SIG"][:], in0=g["SIG"][:], in1=g["CH"][:], op=Alu.min)
            nc.vector.tensor_tensor(out=g["SIG"][:], in0=g["SIG"][:], in1=g["S0"][:], op=Alu.max)

        # ---- Newton update -------------------------------------------
        def newton(gi, width):
            g = grp[gi]
            g_corr(gi, width)
            nc.vector.tensor_scalar(
                out=g["t1"][:], in0=g["GQ"][:], scalar1=FLOOR, scalar2=None, op0=Alu.max,
            )
            nc.vector.reciprocal(g["t1"][:], g["t1"][:])
            nc.vector.tensor_scalar(
                out=g["t5"][:], in0=g["FQ"][:], scalar1=-4.0, scalar2=None, op0=Alu.add,
            )
            nc.vector.tensor_tensor(out=g["t5"][:], in0=g["t5"][:], in1=g["t1"][:], op=Alu.mult)
            nc.vector.tensor_scalar(
                out=g["t5"][:], in0=g["t5"][:], scalar1=0.5, scalar2=None, op0=Alu.mult,
            )
            nc.vector.tensor_tensor(out=g["SIG"][:], in0=g["SIG"][:], in1=g["t5"][:], op=Alu.add)
            nc.vector.tensor_tensor(out=g["SIG"][:], in0=g["SIG"][:], in1=g["CH"][:], op=Alu.min)
            nc.vector.tensor_tensor(out=g["SIG"][:], in0=g["SIG"][:], in1=g["S0"][:], op=Alu.max)

        # ---- final pass + store --------------------------------------
        def stage9(gi):
            g = grp[gi]
            d = FDV[gi]
            dveset = {(i * G) // d + G // (2 * d) for i in range(d)} if d else set()
            for t in range(G):
                ti = gi * G + t
                row0 = ti * P
                v_t = w_pool.tile([P, S], f16, tag="w", name=f"v_{ti}")
                nc.vector.tensor_scalar(
                    out=v_t[:], in0=zs[ti][:],
                    scalar1=g["SIG"][:, t : t + 1], scalar2=g["SIG"][:, t : t + 1],
                    op0=Alu.max, op1=Alu.subtract,
                )
                p_t = p_pool.tile([P, S], f16, tag="p", name=f"p_{ti}")
                if t in dveset:
                    nc.vector.tensor_tensor(
                        out=p_t[:], in0=v_t[:], in1=v_t[:], op=Alu.mult,
                    )
                else:
                    nc.scalar.activation(p_t[:], v_t[:], Act.Square)
                nc.sync.dma_start(out_d[row0 : row0 + P, :], p_t[:])

        def front(gi):
            stage2(gi)
            probe(gi, fcs, WC, r_pool, "rc", 0)
            quad(gi, WC)
            probe(gi, fcs, WC, r_pool, "rc", 1)
            quad(gi, WC)
            probe(gi, fbs, WB, rb_pool, "rb", 2)
            newton(gi, WB)

        # ---- software-pipelined issue order (NG=2) -------------------
        stage1(0)
        front(0)
        stage1(1, 0, 4)
        stage9(0)
        stage1(1, 4, 8)
        front(1)
        stage9(1)

    nc.compile()
    return nc


def _get_program():
    if "nc" not in _CACHE:
        _CACHE["nc"] = _build_program()
    return _CACHE["nc"]


def _prep_z16(scores, mask_b):
    z16 = scores.astype(np.float16)
    np.copyto(z16, np.float16(NEG), where=~mask_b)
    return np.ascontiguousarray(z16)


def _kernel_numpy_fallback(scores, mask, alpha):
    """Reference-equivalent host computation (only for alpha != 1.5)."""
    f32 = np.float32
    alpha = max(float(alpha), 1.0)
    am1 = alpha - 1.0
    x = np.where(mask, scores, f32(-1e9)).astype(f32)
    Xs = (x * f32(am1)).astype(f32)
    mx = Xs.max(axis=-1, keepdims=True)
    tau_lo = mx - f32(1.0)
    tau_hi = mx - f32((1.0 / x.shape[-1]) ** am1)
    dm = tau_hi - tau_lo
    tau_m = tau_lo
    inv = f32(1.0 / am1)
    for _ in range(50):
        dm = dm / 2
        tau_m = tau_lo + dm
        p = np.clip(Xs - tau_m, 0.0, None) ** inv
        f = p.sum(axis=-1, keepdims=True) - 1.0
        tau_lo = np.where(f >= 0, tau_m, tau_lo)
    p = np.clip(Xs - tau_m, 0.0, None) ** inv
    return (p / p.sum(axis=-1, keepdims=True)).astype(f32)


def kernel(scores, mask, alpha):
    scores = np.asarray(scores, dtype=np.float32)
    mask_b = np.asarray(mask).astype(bool)
    alpha_v = float(np.asarray(alpha))

    if abs(max(alpha_v, 1.0) - 1.5) > 1e-6:
        return _kernel_numpy_fallback(scores, mask_b, alpha_v)

    z16 = _prep_z16(scores, mask_b)

    from concourse import bass_utils

    nc = _get_program()
    in_maps = [{"z16": z16[i * BP : (i + 1) * BP]} for i in range(N_CORES)]
    res = bass_utils.run_bass_kernel_spmd(nc, in_maps, core_ids=list(range(N_CORES)))
    outs = []
    for r in res.results:
        p = r["out"].astype(np.float32)
        Z = p.sum(axis=1)
        p /= np.maximum(Z, 1e-9)[:, None]
        outs.append(p)
    return np.concatenate(outs, axis=0)


# revision 10
# speedup vs baseline: 1.4476x; 1.1922x over previous
"""Entmax-1.5 (alpha-entmax via bisection) Trainium2 kernel, v10.

Problem: p = entmax_bisect(where(mask, scores, -1e9), alpha=1.5) over the
last dim of a [16384, 4096] f32 tensor, data-parallel over 8 NeuronCores
(2048 rows per core).

Math: for alpha=1.5, p_i = relu(z_i - tau)^2 / f(tau) with
f(sigma) = sum relu(z - sigma)^2 and f(tau) = 4 at the root.  tau is
located entirely on the pairwise-max TREE levels (fb = max-of-4, 1024
wide; fc = max-of-8, 512 wide), never on the full rows:

  sum relu(level - s)^2 ~= f(s) near the root: each active lane's group
  max survives, only same-group collisions are missed (rare: the active
  set is ~22 of 4096 lanes; max-of-4 collides ~4% of rows).

  sigma0 = C0 + C1*m + C2*mean(fc)                  (regression)
  2x fc-probe  -> frozen-set quadratic in delta form:
                  d = (g - sqrt(g^2 - n(f-4)))/n, n = NS*g^2/f
  1x fb-probe  -> Newton d = (f-4)/(2g)  ->  tau
  final: v = relu(z-tau); p16 = v^2 (fp16); the host divides by the row
  sum (ensure_sum_one exact) and casts to f32 -- the exact
  normalization absorbs the f-error, only tau placement matters.

Each probe is one DVE tensor_scalar (max, stores the clipped row,
accumulates sum -> g after a W*sigma correction) plus one ScalarE
Square(bias) accumulate (-> f), so the DVE-heavy front-end and the
ScalarE work overlap from the first tile.  Inputs fp16 (host folds the
mask; -30 = -inf); output fp16 halves store-side HBM traffic.
Validated vs the jax reference on all 16384 rows: norm_rel ~4.7e-3.
"""

import numpy as np

P = 128          # SBUF partitions
S = 4096         # row length
WB = 1024        # fb width (max-of-4)
WC = 512         # fc width (max-of-8)
MUW = 256        # subsample width for the regression mean
B_FULL = 16384   # total rows
N_CORES = 8
BP = B_FULL // N_CORES   # rows per core
NT = BP // P             # 16 tiles of 128 rows per core
G = 8                    # tiles per stat group
NG = NT // G             # stat groups per core

NEG = -30.0              # mask stand-in for -inf
CLAMP_HI = 0.0312        # tau <= m - 2*sqrt(1/S)
REG = (-0.0227, 0.3391, 0.9736)   # sigma0 = c0 + c1*m + c2*mean(fc)
NS = 1.1                 # n_hat scale in the quadratic
FLOOR = 1e-9

FDV = (0, 3)     # per-group final squares on DVE (tensor_tensor v*v)

_CACHE = {}


def _build_program():
    import concourse.bacc as bacc
    import concourse.tile as tile
    import concourse.mybir as mybir
    from contextlib import ExitStack

    f32 = mybir.dt.float32
    f16 = mybir.dt.float16
    Alu = mybir.AluOpType
    Act = mybir.ActivationFunctionType
    X = mybir.AxisListType.X

    nc = bacc.Bacc(
        "TRN2",
        target_bir_lowering=False,
        debug=False,
        enable_asserts=False,
        num_devices=N_CORES,
    )
    z_d = nc.dram_tensor("z16", [BP, S], f16, kind="ExternalInput").ap()
    out_d = nc.dram_tensor("out", [BP, S], f16, kind="ExternalOutput").ap()

    with tile.TileContext(nc) as tc, ExitStack() as ctx:
        z_pool = ctx.enter_context(tc.tile_pool(name="z", bufs=NT))
        fb_pool = ctx.enter_context(tc.tile_pool(name="fb", bufs=G + 1))
        fc_pool = ctx.enter_context(tc.tile_pool(name="fc", bufs=G + 2))
        f_pool = ctx.enter_context(tc.tile_pool(name="fa", bufs=1))
        r_pool = ctx.enter_context(tc.tile_pool(name="rp", bufs=2))
        rb_pool = ctx.enter_context(tc.tile_pool(name="rb", bufs=2))
        w_pool = ctx.enter_context(tc.tile_pool(name="w", bufs=2))
        p_pool = ctx.enter_context(tc.tile_pool(name="p", bufs=2))
        s_pool = ctx.enter_context(tc.tile_pool(name="st", bufs=2))

        def st(name, gi, dt=f32):
            return s_pool.tile([P, G], dt, tag=name, name=f"{name}_{gi}")

        zs = [None] * NT
        fbs = [None] * NT
        fcs = [None] * NT
        grp = []
        for gi in range(NG):
            g = {}
            grp.append(g)
            for nm in ("M", "CH", "S0", "MU", "SIG", "GA", "FQ", "GQ",
                       "t1", "t2", "t3", "t4", "t5", "t6"):
                g[nm] = st(nm, gi)

        # ---- stage 1: load + max tree + mean(fc) ---------------------
        def stage1(gi, lo=0, hi=G):
            g = grp[gi]
            for t in range(lo, hi):
                ti = gi * G + t
                row0 = ti * P
                z_t = z_pool.tile([P, S], f16, tag="z", name=f"z_{ti}")
                nc.sync.dma_start(z_t[:], z_d[row0 : row0 + P, :])
                zs[ti] = z_t
                fa = f_pool.tile([P, S // 2], f16, tag="fa", name=f"fa_{ti}")
                nc.vector.tensor_tensor(
                    out=fa[:], in0=z_t[:, 0 : S // 2], in1=z_t[:, S // 2 : S],
                    op=Alu.max,
                )
                fb_t = fb_pool.tile([P, WB], f16, tag="fb", name=f"fb_{ti}")
                nc.vector.tensor_tensor(
                    out=fb_t[:], in0=fa[:, 0:WB], in1=fa[:, WB : 2 * WB],
                    op=Alu.max,
                )
                fbs[ti] = fb_t
                fc_t = fc_pool.tile([P, WC], f16, tag="fc", name=f"fc_{ti}")
                nc.vector.tensor_tensor(
                    out=fc_t[:], in0=fb_t[:, 0:WC], in1=fb_t[:, WC : 2 * WC],
                    op=Alu.max,
                )
                fcs[ti] = fc_t
                nc.vector.reduce_max(g["M"][:, t : t + 1], fc_t[:], axis=X)
                j = r_pool.tile([P, MUW], f16, tag="ju", name=f"ju_{ti}")
                nc.scalar.activation(
                    j[:], fc_t[:, 0:MUW], Act.Identity,
                    accum_out=g["MU"][:, t : t + 1],
                )

        # ---- stage 2: regression sigma0 ------------------------------
        def stage2(gi):
            g = grp[gi]
            nc.vector.tensor_scalar(
                out=g["CH"][:], in0=g["M"][:], scalar1=-CLAMP_HI, scalar2=None,
                op0=Alu.add,
            )
            nc.vector.tensor_scalar(
                out=g["S0"][:], in0=g["M"][:], scalar1=-2.0, scalar2=None,
                op0=Alu.add,
            )
            nc.vector.tensor_scalar(
                out=g["t1"][:], in0=g["M"][:], scalar1=REG[1], scalar2=REG[0],
                op0=Alu.mult, op1=Alu.add,
            )
            nc.vector.scalar_tensor_tensor(
                out=g["SIG"][:], in0=g["MU"][:], scalar=REG[2] / MUW,
                in1=g["t1"][:], op0=Alu.mult, op1=Alu.add,
            )
            nc.vector.tensor_tensor(out=g["SIG"][:], in0=g["SIG"][:], in1=g["CH"][:], op=Alu.min)
            nc.vector.tensor_tensor(out=g["SIG"][:], in0=g["SIG"][:], in1=g["S0"][:], op=Alu.max)

        # ---- probe at SIG on a tree level ----------------------------
        def probe(gi, lvls, width, pool, tag, rnd):
            g = grp[gi]
            for t in range(G):
                ti = gi * G + t
                r = pool.tile([P, width], f16, tag=tag, name=f"r{rnd}_{ti}")
                nc.vector.tensor_scalar(
                    out=r[:], in0=lvls[ti][:],
                    scalar1=g["SIG"][:, t : t + 1], scalar2=None,
                    op0=Alu.max, op1=Alu.add,
                    accum_out=g["GA"][:, t : t + 1],
                )
                nc.scalar.activation(
                    r[:], r[:], Act.Square,
                    bias=g["SIG"][:, t : t + 1], scale=-1.0,
                    accum_out=g["FQ"][:, t : t + 1],
                )

        def g_corr(gi, width):
            g = grp[gi]
            nc.vector.scalar_tensor_tensor(
                out=g["GQ"][:], in0=g["SIG"][:], scalar=-float(width),
                in1=g["GA"][:], op0=Alu.mult, op1=Alu.add,
            )

        # ---- quadratic update (delta form) ---------------------------
        def quad(gi, width):
            g = grp[gi]
            g_corr(gi, width)
            nc.vector.tensor_scalar(
                out=g["t2"][:], in0=g["FQ"][:], scalar1=FLOOR, scalar2=None, op0=Alu.max,
            )
            nc.vector.tensor_tensor(out=g["t6"][:], in0=g["GQ"][:], in1=g["GQ"][:], op=Alu.mult)
            nc.vector.reciprocal(g["t4"][:], g["t2"][:])
            nc.vector.tensor_tensor(out=g["t3"][:], in0=g["t6"][:], in1=g["t4"][:], op=Alu.mult)
            nc.vector.tensor_scalar(
                out=g["t3"][:], in0=g["t3"][:], scalar1=NS, scalar2=1.0,
                op0=Alu.mult, op1=Alu.max,
            )
            nc.vector.tensor_scalar(
                out=g["t5"][:], in0=g["FQ"][:], scalar1=-4.0, scalar2=None, op0=Alu.add,
            )
            nc.vector.tensor_tensor(out=g["t5"][:], in0=g["t3"][:], in1=g["t5"][:], op=Alu.mult)
            nc.vector.tensor_tensor(out=g["t5"][:], in0=g["t6"][:], in1=g["t5"][:], op=Alu.subtract)
            nc.vector.tensor_scalar(
                out=g["t5"][:], in0=g["t5"][:], scalar1=0.0, scalar2=None, op0=Alu.max,
            )
            nc.scalar.activation(g["t5"][:], g["t5"][:], Act.Sqrt)
            nc.vector.tensor_tensor(out=g["t5"][:], in0=g["GQ"][:], in1=g["t5"][:], op=Alu.subtract)
            nc.vector.reciprocal(g["t4"][:], g["t3"][:])
            nc.vector.tensor_tensor(out=g["t5"][:], in0=g["t5"][:], in1=g["t4"][:], op=Alu.mult)
            nc.vector.tensor_tensor(out=g["SIG"][:], in0=g["SIG"][:], in1=g["t5"][:], op=Alu.add)
            nc.vector.tensor_tensor(out=g["SIG"][:], in0=g["SIG"][:], in1=g["CH"][:], op=Alu.min)
            nc.vector.tensor_tensor(out=g["SIG"][:], in0=g["SIG"][:], in1=g["S0"][:], op=Alu.max)

        # ---- Newton update -------------------------------------------
        def newton(gi, width):
            g = grp[gi]
            g_corr(gi, width)
            nc.vector.tensor_scalar(
                out=g["t1"][:], in0=g["GQ"][:], scalar1=FLOOR, scalar2=None, op0=Alu.max,
            )
            nc.vector.reciprocal(g["t1"][:], g["t1"][:])
            nc.vector.tensor_scalar(
                out=g["t5"][:], in0=g["FQ"][:], scalar1=-4.0, scalar2=None, op0=Alu.add,
            )
            nc.vector.tensor_tensor(out=g["t5"][:], in0=g["t5"][:], in1=g["t1"][:], op=Alu.mult)
            nc.vector.tensor_scalar(
                out=g["t5"][:], in0=g["t5"][:], scalar1=0.5, scalar2=None, op0=Alu.mult,
            )
            nc.vector.tensor_tensor(out=g["SIG"][:], in0=g["SIG"][:], in1=g["t5"][:], op=Alu.add)
            nc.vector.tensor_tensor(out=g["SIG"][:], in0=g["SIG"][:], in1=g["CH"][:], op=Alu.min)
            nc.vector.tensor_tensor(out=g["SIG"][:], in0=g["SIG"][:], in1=g["S0"][:], op=Alu.max)

        # ---- final pass + store --------------------------------------
        def stage9(gi):
            g = grp[gi]
            d = FDV[gi]
            dveset = {(i * G) // d + G // (2 * d) for i in range(d)} if d else set()
            for t in range(G):
                ti = gi * G + t
                row0 = ti * P
                v_t = w_pool.tile([P, S], f16, tag="w", name=f"v_{ti}")
                nc.vector.tensor_scalar(
                    out=v_t[:], in0=zs[ti][:],
                    scalar1=g["SIG"][:, t : t + 1], scalar2=g["SIG"][:, t : t + 1],
                    op0=Alu.max, op1=Alu.subtract,
                )
                p_t = p_pool.tile([P, S], f16, tag="p", name=f"p_{ti}")
                if t in dveset:
                    nc.vector.tensor_tensor(
                        out=p_t[:], in0=v_t[:], in1=v_t[:], op=Alu.mult,
                    )
                else:
                    nc.scalar.activation(p_t[:], v_t[:], Act.Square)
                nc.sync.dma_start(out_d[row0 : row0 + P, :], p_t[:])

        def front(gi):
            stage2(gi)
            probe(gi, fcs, WC, r_pool, "rc", 0)
            quad(gi, WC)
            probe(gi, fcs, WC, r_pool, "rc", 1)
            newton(gi, WC)
            probe(gi, fbs, WB, rb_pool, "rb", 2)
            newton(gi, WB)

        # ---- software-pipelined issue order (NG=2) -------------------
        stage1(0)
        front(0)
        stage9(0)
        stage1(1)
        front(1)
        stage9(1)

    nc.compile()
    return nc


def _get_program():
    if "nc" not in _CACHE:
        _CACHE["nc"] = _build_program()
    return _CACHE["nc"]


def _prep_z16(scores, mask_b):
    z16 = scores.astype(np.float16)
    np.copyto(z16, np.float16(NEG), where=~mask_b)
    return np.ascontiguousarray(z16)


def _kernel_numpy_fallback(scores, mask, alpha):
    """Reference-equivalent host computation (only for alpha != 1.5)."""
    f32 = np.float32
    alpha = max(float(alpha), 1.0)
    am1 = alpha - 1.0
    x = np.where(mask, scores, f32(-1e9)).astype(f32)
    Xs = (x * f32(am1)).astype(f32)
    mx = Xs.max(axis=-1, keepdims=True)
    tau_lo = mx - f32(1.0)
    tau_hi = mx - f32((1.0 / x.shape[-1]) ** am1)
    dm = tau_hi - tau_lo
    tau_m = tau_lo
    inv = f32(1.0 / am1)
    for _ in range(50):
        dm = dm / 2
        tau_m = tau_lo + dm
        p = np.clip(Xs - tau_m, 0.0, None) ** inv
        f = p.sum(axis=-1, keepdims=True) - 1.0
        tau_lo = np.where(f >= 0, tau_m, tau_lo)
    p = np.clip(Xs - tau_m, 0.0, None) ** inv
    return (p / p.sum(axis=-1, keepdims=True)).astype(f32)


def kernel(scores, mask, alpha):
    scores = np.asarray(scores, dtype=np.float32)
    mask_b = np.asarray(mask).astype(bool)
    alpha_v = float(np.asarray(alpha))

    if abs(max(alpha_v, 1.0) - 1.5) > 1e-6:
        return _kernel_numpy_fallback(scores, mask_b, alpha_v)

    z16 = _prep_z16(scores, mask_b)

    from concourse import bass_utils

    nc = _get_program()
    in_maps = [{"z16": z16[i * BP : (i + 1) * BP]} for i in range(N_CORES)]
    res = bass_utils.run_bass_kernel_spmd(nc, in_maps, core_ids=list(range(N_CORES)))
    outs = []
    for r in res.results:
        p = r["out"].astype(np.float32)
        Z = p.sum(axis=1)
        p /= np.maximum(Z, 1e-9)[:, None]
        outs.append(p)
    return np.concatenate(outs, axis=0)


# revision 11
# speedup vs baseline: 1.4736x; 1.0179x over previous
"""Entmax-1.5 (alpha-entmax via bisection) Trainium2 kernel, v10.

Problem: p = entmax_bisect(where(mask, scores, -1e9), alpha=1.5) over the
last dim of a [16384, 4096] f32 tensor, data-parallel over 8 NeuronCores
(2048 rows per core).

Math: for alpha=1.5, p_i = relu(z_i - tau)^2 / f(tau) with
f(sigma) = sum relu(z - sigma)^2 and f(tau) = 4 at the root.  tau is
located entirely on the pairwise-max TREE levels (fb = max-of-4, 1024
wide; fc = max-of-8, 512 wide), never on the full rows:

  sum relu(level - s)^2 ~= f(s) near the root: each active lane's group
  max survives, only same-group collisions are missed (rare: the active
  set is ~22 of 4096 lanes; max-of-4 collides ~4% of rows).

  sigma0 = C0 + C1*m + C2*mean(fc)                  (regression)
  2x fc-probe  -> frozen-set quadratic in delta form:
                  d = (g - sqrt(g^2 - n(f-4)))/n, n = NS*g^2/f
  1x fb-probe  -> Newton d = (f-4)/(2g)  ->  tau
  final: v = relu(z-tau); p16 = v^2 (fp16); the host divides by the row
  sum (ensure_sum_one exact) and casts to f32 -- the exact
  normalization absorbs the f-error, only tau placement matters.

Each probe is one DVE tensor_scalar (max, stores the clipped row,
accumulates sum -> g after a W*sigma correction) plus one ScalarE
Square(bias) accumulate (-> f), so the DVE-heavy front-end and the
ScalarE work overlap from the first tile.  Inputs fp16 (host folds the
mask; -30 = -inf); output fp16 halves store-side HBM traffic.
Validated vs the jax reference on all 16384 rows: norm_rel ~4.7e-3.
"""

import numpy as np

P = 128          # SBUF partitions
S = 4096         # row length
WB = 1024        # fb width (max-of-4)
WC = 512         # fc width (max-of-8)
MUW = 256        # subsample width for the regression mean
B_FULL = 16384   # total rows
N_CORES = 8
BP = B_FULL // N_CORES   # rows per core
NT = BP // P             # 16 tiles of 128 rows per core
G = 8                    # tiles per stat group
NG = NT // G             # stat groups per core

NEG = -30.0              # mask stand-in for -inf
CLAMP_HI = 0.0312        # tau <= m - 2*sqrt(1/S)
REG = (-0.0227, 0.3391, 0.9736)   # sigma0 = c0 + c1*m + c2*mean(fc)
NS = 1.1                 # n_hat scale in the quadratic
FLOOR = 1e-9

FDV = (0, 3)     # per-group final squares on DVE (tensor_tensor v*v)
ACT_V = (5, 0)   # per-group final v-relus on ScalarE (Act Relu w/ bias)

_CACHE = {}


def _build_program():
    import concourse.bacc as bacc
    import concourse.tile as tile
    import concourse.mybir as mybir
    from contextlib import ExitStack

    f32 = mybir.dt.float32
    f16 = mybir.dt.float16
    Alu = mybir.AluOpType
    Act = mybir.ActivationFunctionType
    X = mybir.AxisListType.X

    nc = bacc.Bacc(
        "TRN2",
        target_bir_lowering=False,
        debug=False,
        enable_asserts=False,
        num_devices=N_CORES,
    )
    z_d = nc.dram_tensor("z16", [BP, S], f16, kind="ExternalInput").ap()
    out_d = nc.dram_tensor("out", [BP, S], f16, kind="ExternalOutput").ap()

    with tile.TileContext(nc) as tc, ExitStack() as ctx:
        z_pool = ctx.enter_context(tc.tile_pool(name="z", bufs=NT))
        fb_pool = ctx.enter_context(tc.tile_pool(name="fb", bufs=G + 1))
        fc_pool = ctx.enter_context(tc.tile_pool(name="fc", bufs=G + 2))
        f_pool = ctx.enter_context(tc.tile_pool(name="fa", bufs=1))
        r_pool = ctx.enter_context(tc.tile_pool(name="rp", bufs=2))
        rb_pool = ctx.enter_context(tc.tile_pool(name="rb", bufs=2))
        w_pool = ctx.enter_context(tc.tile_pool(name="w", bufs=2))
        p_pool = ctx.enter_context(tc.tile_pool(name="p", bufs=2))
        s_pool = ctx.enter_context(tc.tile_pool(name="st", bufs=2))

        def st(name, gi, dt=f32):
            return s_pool.tile([P, G], dt, tag=name, name=f"{name}_{gi}")

        zs = [None] * NT
        fbs = [None] * NT
        fcs = [None] * NT
        grp = []
        for gi in range(NG):
            g = {}
            grp.append(g)
            for nm in ("M", "CH", "S0", "MU", "SIG", "NTAU", "GA", "FQ", "GQ",
                       "t1", "t2", "t3", "t4", "t5", "t6"):
                g[nm] = st(nm, gi)

        # ---- stage 1: load + max tree + mean(fc) ---------------------
        def stage1(gi, lo=0, hi=G):
            g = grp[gi]
            for t in range(lo, hi):
                ti = gi * G + t
                row0 = ti * P
                z_t = z_pool.tile([P, S], f16, tag="z", name=f"z_{ti}")
                nc.sync.dma_start(z_t[:], z_d[row0 : row0 + P, :])
                zs[ti] = z_t
                fa = f_pool.tile([P, S // 2], f16, tag="fa", name=f"fa_{ti}")
                nc.vector.tensor_tensor(
                    out=fa[:], in0=z_t[:, 0 : S // 2], in1=z_t[:, S // 2 : S],
                    op=Alu.max,
                )
                fb_t = fb_pool.tile([P, WB], f16, tag="fb", name=f"fb_{ti}")
                nc.vector.tensor_tensor(
                    out=fb_t[:], in0=fa[:, 0:WB], in1=fa[:, WB : 2 * WB],
                    op=Alu.max,
                )
                fbs[ti] = fb_t
                fc_t = fc_pool.tile([P, WC], f16, tag="fc", name=f"fc_{ti}")
                nc.vector.tensor_tensor(
                    out=fc_t[:], in0=fb_t[:, 0:WC], in1=fb_t[:, WC : 2 * WC],
                    op=Alu.max,
                )
                fcs[ti] = fc_t
                nc.vector.reduce_max(g["M"][:, t : t + 1], fc_t[:], axis=X)
                j = r_pool.tile([P, MUW], f16, tag="ju", name=f"ju_{ti}")
                nc.scalar.activation(
                    j[:], fc_t[:, 0:MUW], Act.Identity,
                    accum_out=g["MU"][:, t : t + 1],
                )

        # ---- stage 2: regression sigma0 ------------------------------
        def stage2(gi):
            g = grp[gi]
            nc.vector.tensor_scalar(
                out=g["CH"][:], in0=g["M"][:], scalar1=-CLAMP_HI, scalar2=None,
                op0=Alu.add,
            )
            nc.vector.tensor_scalar(
                out=g["S0"][:], in0=g["M"][:], scalar1=-2.0, scalar2=None,
                op0=Alu.add,
            )
            nc.vector.tensor_scalar(
                out=g["t1"][:], in0=g["M"][:], scalar1=REG[1], scalar2=REG[0],
                op0=Alu.mult, op1=Alu.add,
            )
            nc.vector.scalar_tensor_tensor(
                out=g["SIG"][:], in0=g["MU"][:], scalar=REG[2] / MUW,
                in1=g["t1"][:], op0=Alu.mult, op1=Alu.add,
            )
            nc.vector.tensor_tensor(out=g["SIG"][:], in0=g["SIG"][:], in1=g["CH"][:], op=Alu.min)
            nc.vector.tensor_tensor(out=g["SIG"][:], in0=g["SIG"][:], in1=g["S0"][:], op=Alu.max)

        # ---- probe at SIG on a tree level ----------------------------
        def probe(gi, lvls, width, pool, tag, rnd):
            g = grp[gi]
            for t in range(G):
                ti = gi * G + t
                r = pool.tile([P, width], f16, tag=tag, name=f"r{rnd}_{ti}")
                nc.vector.tensor_scalar(
                    out=r[:], in0=lvls[ti][:],
                    scalar1=g["SIG"][:, t : t + 1], scalar2=None,
                    op0=Alu.max, op1=Alu.add,
                    accum_out=g["GA"][:, t : t + 1],
                )
                nc.scalar.activation(
                    r[:], r[:], Act.Square,
                    bias=g["SIG"][:, t : t + 1], scale=-1.0,
                    accum_out=g["FQ"][:, t : t + 1],
                )

        def g_corr(gi, width):
            g = grp[gi]
            nc.vector.scalar_tensor_tensor(
                out=g["GQ"][:], in0=g["SIG"][:], scalar=-float(width),
                in1=g["GA"][:], op0=Alu.mult, op1=Alu.add,
            )

        # ---- quadratic update (delta form) ---------------------------
        def quad(gi, width):
            g = grp[gi]
            g_corr(gi, width)
            nc.vector.tensor_scalar(
                out=g["t2"][:], in0=g["FQ"][:], scalar1=FLOOR, scalar2=None, op0=Alu.max,
            )
            nc.vector.tensor_tensor(out=g["t6"][:], in0=g["GQ"][:], in1=g["GQ"][:], op=Alu.mult)
            nc.vector.reciprocal(g["t4"][:], g["t2"][:])
            nc.vector.tensor_tensor(out=g["t3"][:], in0=g["t6"][:], in1=g["t4"][:], op=Alu.mult)
            nc.vector.tensor_scalar(
                out=g["t3"][:], in0=g["t3"][:], scalar1=NS, scalar2=1.0,
                op0=Alu.mult, op1=Alu.max,
            )
            nc.vector.tensor_scalar(
                out=g["t5"][:], in0=g["FQ"][:], scalar1=-4.0, scalar2=None, op0=Alu.add,
            )
            nc.vector.tensor_tensor(out=g["t5"][:], in0=g["t3"][:], in1=g["t5"][:], op=Alu.mult)
            nc.vector.tensor_tensor(out=g["t5"][:], in0=g["t6"][:], in1=g["t5"][:], op=Alu.subtract)
            nc.vector.tensor_scalar(
                out=g["t5"][:], in0=g["t5"][:], scalar1=0.0, scalar2=None, op0=Alu.max,
            )
            nc.scalar.activation(g["t5"][:], g["t5"][:], Act.Sqrt)
            nc.vector.tensor_tensor(out=g["t5"][:], in0=g["GQ"][:], in1=g["t5"][:], op=Alu.subtract)
            nc.vector.reciprocal(g["t4"][:], g["t3"][:])
            nc.vector.tensor_tensor(out=g["t5"][:], in0=g["t5"][:], in1=g["t4"][:], op=Alu.mult)
            nc.vector.tensor_tensor(out=g["SIG"][:], in0=g["SIG"][:], in1=g["t5"][:], op=Alu.add)
            nc.vector.tensor_tensor(out=g["SIG"][:], in0=g["SIG"][:], in1=g["CH"][:], op=Alu.min)
            nc.vector.tensor_tensor(out=g["SIG"][:], in0=g["SIG"][:], in1=g["S0"][:], op=Alu.max)

        # ---- Newton update -------------------------------------------
        def newton(gi, width):
            g = grp[gi]
            g_corr(gi, width)
            nc.vector.tensor_scalar(
                out=g["t1"][:], in0=g["GQ"][:], scalar1=FLOOR, scalar2=None, op0=Alu.max,
            )
            nc.vector.reciprocal(g["t1"][:], g["t1"][:])
            nc.vector.tensor_scalar(
                out=g["t5"][:], in0=g["FQ"][:], scalar1=-4.0, scalar2=None, op0=Alu.add,
            )
            nc.vector.tensor_tensor(out=g["t5"][:], in0=g["t5"][:], in1=g["t1"][:], op=Alu.mult)
            nc.vector.tensor_scalar(
                out=g["t5"][:], in0=g["t5"][:], scalar1=0.5, scalar2=None, op0=Alu.mult,
            )
            nc.vector.tensor_tensor(out=g["SIG"][:], in0=g["SIG"][:], in1=g["t5"][:], op=Alu.add)
            nc.vector.tensor_tensor(out=g["SIG"][:], in0=g["SIG"][:], in1=g["CH"][:], op=Alu.min)
            nc.vector.tensor_tensor(out=g["SIG"][:], in0=g["SIG"][:], in1=g["S0"][:], op=Alu.max)
            if ACT_V[gi] > 0:
                nc.vector.tensor_scalar(
                    out=g["NTAU"][:], in0=g["SIG"][:], scalar1=-1.0, scalar2=None,
                    op0=Alu.mult,
                )

        # ---- final pass + store --------------------------------------
        def stage9(gi):
            g = grp[gi]
            d = FDV[gi]
            dveset = {(i * G) // d + G // (2 * d) for i in range(d)} if d else set()
            a = ACT_V[gi]
            actset = {(i * G) // a + G // (2 * a) for i in range(a)} if a else set()
            for t in range(G):
                ti = gi * G + t
                row0 = ti * P
                v_t = w_pool.tile([P, S], f16, tag="w", name=f"v_{ti}")
                if t in actset:
                    nc.scalar.activation(
                        v_t[:], zs[ti][:], Act.Relu, bias=g["NTAU"][:, t : t + 1],
                    )
                else:
                    nc.vector.tensor_scalar(
                        out=v_t[:], in0=zs[ti][:],
                        scalar1=g["SIG"][:, t : t + 1], scalar2=g["SIG"][:, t : t + 1],
                        op0=Alu.max, op1=Alu.subtract,
                    )
                p_t = p_pool.tile([P, S], f16, tag="p", name=f"p_{ti}")
                if t in dveset:
                    nc.vector.tensor_tensor(
                        out=p_t[:], in0=v_t[:], in1=v_t[:], op=Alu.mult,
                    )
                else:
                    nc.scalar.activation(p_t[:], v_t[:], Act.Square)
                nc.sync.dma_start(out_d[row0 : row0 + P, :], p_t[:])

        def front(gi):
            stage2(gi)
            probe(gi, fcs, WC, r_pool, "rc", 0)
            quad(gi, WC)
            probe(gi, fbs, WB, rb_pool, "rb", 2)
            newton(gi, WB)

        # ---- software-pipelined issue order (NG=2) -------------------
        stage1(0)
        front(0)
        stage9(0)
        stage1(1)
        front(1)
        stage9(1)

    nc.compile()
    return nc


def _get_program():
    if "nc" not in _CACHE:
        _CACHE["nc"] = _build_program()
    return _CACHE["nc"]


def _prep_z16(scores, mask_b):
    z16 = scores.astype(np.float16)
    np.copyto(z16, np.float16(NEG), where=~mask_b)
    return np.ascontiguousarray(z16)


def _kernel_numpy_fallback(scores, mask, alpha):
    """Reference-equivalent host computation (only for alpha != 1.5)."""
    f32 = np.float32
    alpha = max(float(alpha), 1.0)
    am1 = alpha - 1.0
    x = np.where(mask, scores, f32(-1e9)).astype(f32)
    Xs = (x * f32(am1)).astype(f32)
    mx = Xs.max(axis=-1, keepdims=True)
    tau_lo = mx - f32(1.0)
    tau_hi = mx - f32((1.0 / x.shape[-1]) ** am1)
    dm = tau_hi - tau_lo
    tau_m = tau_lo
    inv = f32(1.0 / am1)
    for _ in range(50):
        dm = dm / 2
        tau_m = tau_lo + dm
        p = np.clip(Xs - tau_m, 0.0, None) ** inv
        f = p.sum(axis=-1, keepdims=True) - 1.0
        tau_lo = np.where(f >= 0, tau_m, tau_lo)
    p = np.clip(Xs - tau_m, 0.0, None) ** inv
    return (p / p.sum(axis=-1, keepdims=True)).astype(f32)


def kernel(scores, mask, alpha):
    scores = np.asarray(scores, dtype=np.float32)
    mask_b = np.asarray(mask).astype(bool)
    alpha_v = float(np.asarray(alpha))

    if abs(max(alpha_v, 1.0) - 1.5) > 1e-6:
        return _kernel_numpy_fallback(scores, mask_b, alpha_v)

    z16 = _prep_z16(scores, mask_b)

    from concourse import bass_utils

    nc = _get_program()
    in_maps = [{"z16": z16[i * BP : (i + 1) * BP]} for i in range(N_CORES)]
    res = bass_utils.run_bass_kernel_spmd(nc, in_maps, core_ids=list(range(N_CORES)))
    outs = []
    for r in res.results:
        p = r["out"].astype(np.float32)
        Z = p.sum(axis=1)
        p /= np.maximum(Z, 1e-9)[:, None]
        outs.append(p)
    return np.concatenate(outs, axis=0)


# revision 12
# speedup vs baseline: 1.6076x; 1.0909x over previous
"""Entmax-1.5 (alpha-entmax via bisection) Trainium2 kernel, v10.

Problem: p = entmax_bisect(where(mask, scores, -1e9), alpha=1.5) over the
last dim of a [16384, 4096] f32 tensor, data-parallel over 8 NeuronCores
(2048 rows per core).

Math: for alpha=1.5, p_i = relu(z_i - tau)^2 / f(tau) with
f(sigma) = sum relu(z - sigma)^2 and f(tau) = 4 at the root.  tau is
located entirely on the pairwise-max TREE levels (fb = max-of-4, 1024
wide; fc = max-of-8, 512 wide), never on the full rows:

  sum relu(level - s)^2 ~= f(s) near the root: each active lane's group
  max survives, only same-group collisions are missed (rare: the active
  set is ~22 of 4096 lanes; max-of-4 collides ~4% of rows).

  sigma0 = C0 + C1*m + C2*mean(fc)                  (regression)
  2x fc-probe  -> frozen-set quadratic in delta form:
                  d = (g - sqrt(g^2 - n(f-4)))/n, n = NS*g^2/f
  1x fb-probe  -> Newton d = (f-4)/(2g)  ->  tau
  final: v = relu(z-tau); p16 = v^2 (fp16); the host divides by the row
  sum (ensure_sum_one exact) and casts to f32 -- the exact
  normalization absorbs the f-error, only tau placement matters.

Each probe is one DVE tensor_scalar (max, stores the clipped row,
accumulates sum -> g after a W*sigma correction) plus one ScalarE
Square(bias) accumulate (-> f), so the DVE-heavy front-end and the
ScalarE work overlap from the first tile.  Inputs fp16 (host folds the
mask; -30 = -inf); output fp16 halves store-side HBM traffic.
Validated vs the jax reference on all 16384 rows: norm_rel ~4.7e-3.
"""

import numpy as np

P = 128          # SBUF partitions
S = 4096         # row length
WB = 1024        # fb width (max-of-4)
WC = 512         # fc width (max-of-8)
MUW = 256        # subsample width for the regression mean
B_FULL = 16384   # total rows
N_CORES = 8
BP = B_FULL // N_CORES   # rows per core
NT = BP // P             # 16 tiles of 128 rows per core
G = 8                    # tiles per stat group
NG = NT // G             # stat groups per core

NEG = -30.0              # mask stand-in for -inf
CLAMP_HI = 0.0312        # tau <= m - 2*sqrt(1/S)
REG = (-0.0227, 0.3391, 0.9736)   # sigma0 = c0 + c1*m + c2*mean(fc)
NS = 1.1                 # n_hat scale in the quadratic
FLOOR = 1e-9

FDV = (0, 4)     # per-group final squares on DVE (tensor_tensor v*v)
ACT_V = (0, 2)   # per-group final v-relus on ScalarE (Act Relu w/ bias)

_CACHE = {}


def _build_program():
    import concourse.bacc as bacc
    import concourse.tile as tile
    import concourse.mybir as mybir
    from contextlib import ExitStack

    f32 = mybir.dt.float32
    f16 = mybir.dt.float16
    Alu = mybir.AluOpType
    Act = mybir.ActivationFunctionType
    X = mybir.AxisListType.X

    nc = bacc.Bacc(
        "TRN2",
        target_bir_lowering=False,
        debug=False,
        enable_asserts=False,
        num_devices=N_CORES,
    )
    z_d = nc.dram_tensor("z16", [BP, S], f16, kind="ExternalInput").ap()
    out_d = nc.dram_tensor("out", [BP, S], f16, kind="ExternalOutput").ap()

    with tile.TileContext(nc) as tc, ExitStack() as ctx:
        z_pool = ctx.enter_context(tc.tile_pool(name="z", bufs=NT))
        fb_pool = ctx.enter_context(tc.tile_pool(name="fb", bufs=G + 1))
        fc_pool = ctx.enter_context(tc.tile_pool(name="fc", bufs=G + 2))
        f_pool = ctx.enter_context(tc.tile_pool(name="fa", bufs=1))
        r_pool = ctx.enter_context(tc.tile_pool(name="rp", bufs=2))
        rb_pool = ctx.enter_context(tc.tile_pool(name="rb", bufs=2))
        w_pool = ctx.enter_context(tc.tile_pool(name="w", bufs=2))
        p_pool = ctx.enter_context(tc.tile_pool(name="p", bufs=2))
        s_pool = ctx.enter_context(tc.tile_pool(name="st", bufs=2))

        def st(name, gi, dt=f32):
            return s_pool.tile([P, G], dt, tag=name, name=f"{name}_{gi}")

        zs = [None] * NT
        fbs = [None] * NT
        fcs = [None] * NT
        grp = []
        for gi in range(NG):
            g = {}
            grp.append(g)
            for nm in ("M", "CH", "S0", "MU", "SIG", "NTAU", "GA", "FQ", "GQ",
                       "t1", "t2", "t3", "t4", "t5", "t6"):
                g[nm] = st(nm, gi)

        # ---- stage 1: load + max tree + mean(fc) ---------------------
        def stage1(gi, lo=0, hi=G):
            g = grp[gi]
            for t in range(lo, hi):
                ti = gi * G + t
                row0 = ti * P
                z_t = z_pool.tile([P, S], f16, tag="z", name=f"z_{ti}")
                nc.sync.dma_start(z_t[:], z_d[row0 : row0 + P, :])
                zs[ti] = z_t
                fa = f_pool.tile([P, S // 2], f16, tag="fa", name=f"fa_{ti}")
                nc.vector.tensor_tensor(
                    out=fa[:], in0=z_t[:, 0 : S // 2], in1=z_t[:, S // 2 : S],
                    op=Alu.max,
                )
                fb_t = fb_pool.tile([P, WB], f16, tag="fb", name=f"fb_{ti}")
                nc.vector.tensor_tensor(
                    out=fb_t[:], in0=fa[:, 0:WB], in1=fa[:, WB : 2 * WB],
                    op=Alu.max,
                )
                fbs[ti] = fb_t
                fc_t = fc_pool.tile([P, WC], f16, tag="fc", name=f"fc_{ti}")
                nc.vector.tensor_tensor(
                    out=fc_t[:], in0=fb_t[:, 0:WC], in1=fb_t[:, WC : 2 * WC],
                    op=Alu.max,
                )
                fcs[ti] = fc_t
                nc.vector.reduce_max(g["M"][:, t : t + 1], fc_t[:], axis=X)
                j = r_pool.tile([P, MUW], f16, tag="ju", name=f"ju_{ti}")
                nc.scalar.activation(
                    j[:], fc_t[:, 0:MUW], Act.Identity,
                    accum_out=g["MU"][:, t : t + 1],
                )

        # ---- stage 2: regression sigma0 ------------------------------
        def stage2(gi):
            g = grp[gi]
            nc.vector.tensor_scalar(
                out=g["CH"][:], in0=g["M"][:], scalar1=-CLAMP_HI, scalar2=None,
                op0=Alu.add,
            )
            nc.vector.tensor_scalar(
                out=g["S0"][:], in0=g["M"][:], scalar1=-2.0, scalar2=None,
                op0=Alu.add,
            )
            nc.vector.tensor_scalar(
                out=g["t1"][:], in0=g["M"][:], scalar1=REG[1], scalar2=REG[0],
                op0=Alu.mult, op1=Alu.add,
            )
            nc.vector.scalar_tensor_tensor(
                out=g["SIG"][:], in0=g["MU"][:], scalar=REG[2] / MUW,
                in1=g["t1"][:], op0=Alu.mult, op1=Alu.add,
            )
            nc.vector.tensor_tensor(out=g["SIG"][:], in0=g["SIG"][:], in1=g["CH"][:], op=Alu.min)
            nc.vector.tensor_tensor(out=g["SIG"][:], in0=g["SIG"][:], in1=g["S0"][:], op=Alu.max)

        # ---- probe at SIG on a tree level ----------------------------
        def probe(gi, lvls, width, pool, tag, rnd):
            g = grp[gi]
            for t in range(G):
                ti = gi * G + t
                r = pool.tile([P, width], f16, tag=tag, name=f"r{rnd}_{ti}")
                nc.vector.tensor_scalar(
                    out=r[:], in0=lvls[ti][:],
                    scalar1=g["SIG"][:, t : t + 1], scalar2=None,
                    op0=Alu.max, op1=Alu.add,
                    accum_out=g["GA"][:, t : t + 1],
                )
                nc.scalar.activation(
                    r[:], r[:], Act.Square,
                    bias=g["SIG"][:, t : t + 1], scale=-1.0,
                    accum_out=g["FQ"][:, t : t + 1],
                )

        def g_corr(gi, width):
            g = grp[gi]
            nc.vector.scalar_tensor_tensor(
                out=g["GQ"][:], in0=g["SIG"][:], scalar=-float(width),
                in1=g["GA"][:], op0=Alu.mult, op1=Alu.add,
            )

        # ---- quadratic update (delta form) ---------------------------
        def quad(gi, width):
            g = grp[gi]
            g_corr(gi, width)
            nc.vector.tensor_scalar(
                out=g["t2"][:], in0=g["FQ"][:], scalar1=FLOOR, scalar2=None, op0=Alu.max,
            )
            nc.vector.tensor_tensor(out=g["t6"][:], in0=g["GQ"][:], in1=g["GQ"][:], op=Alu.mult)
            nc.vector.reciprocal(g["t4"][:], g["t2"][:])
            nc.vector.tensor_tensor(out=g["t3"][:], in0=g["t6"][:], in1=g["t4"][:], op=Alu.mult)
            nc.vector.tensor_scalar(
                out=g["t3"][:], in0=g["t3"][:], scalar1=NS, scalar2=1.0,
                op0=Alu.mult, op1=Alu.max,
            )
            nc.vector.tensor_scalar(
                out=g["t5"][:], in0=g["FQ"][:], scalar1=-4.0, scalar2=None, op0=Alu.add,
            )
            nc.vector.tensor_tensor(out=g["t5"][:], in0=g["t3"][:], in1=g["t5"][:], op=Alu.mult)
            nc.vector.tensor_tensor(out=g["t5"][:], in0=g["t6"][:], in1=g["t5"][:], op=Alu.subtract)
            nc.vector.tensor_scalar(
                out=g["t5"][:], in0=g["t5"][:], scalar1=0.0, scalar2=None, op0=Alu.max,
            )
            nc.scalar.activation(g["t5"][:], g["t5"][:], Act.Sqrt)
            nc.vector.tensor_tensor(out=g["t5"][:], in0=g["GQ"][:], in1=g["t5"][:], op=Alu.subtract)
            nc.vector.reciprocal(g["t4"][:], g["t3"][:])
            nc.vector.tensor_tensor(out=g["t5"][:], in0=g["t5"][:], in1=g["t4"][:], op=Alu.mult)
            nc.vector.tensor_tensor(out=g["SIG"][:], in0=g["SIG"][:], in1=g["t5"][:], op=Alu.add)
            nc.vector.tensor_tensor(out=g["SIG"][:], in0=g["SIG"][:], in1=g["CH"][:], op=Alu.min)
            nc.vector.tensor_tensor(out=g["SIG"][:], in0=g["SIG"][:], in1=g["S0"][:], op=Alu.max)

        # ---- Newton update -------------------------------------------
        def newton(gi, width):
            g = grp[gi]
            g_corr(gi, width)
            nc.vector.tensor_scalar(
                out=g["t1"][:], in0=g["GQ"][:], scalar1=FLOOR, scalar2=None, op0=Alu.max,
            )
            nc.vector.reciprocal(g["t1"][:], g["t1"][:])
            nc.vector.tensor_scalar(
                out=g["t5"][:], in0=g["FQ"][:], scalar1=-4.0, scalar2=None, op0=Alu.add,
            )
            nc.vector.tensor_tensor(out=g["t5"][:], in0=g["t5"][:], in1=g["t1"][:], op=Alu.mult)
            nc.vector.tensor_scalar(
                out=g["t5"][:], in0=g["t5"][:], scalar1=0.5, scalar2=None, op0=Alu.mult,
            )
            nc.vector.tensor_tensor(out=g["SIG"][:], in0=g["SIG"][:], in1=g["t5"][:], op=Alu.add)
            nc.vector.tensor_tensor(out=g["SIG"][:], in0=g["SIG"][:], in1=g["CH"][:], op=Alu.min)
            nc.vector.tensor_tensor(out=g["SIG"][:], in0=g["SIG"][:], in1=g["S0"][:], op=Alu.max)
            if ACT_V[gi] > 0:
                nc.vector.tensor_scalar(
                    out=g["NTAU"][:], in0=g["SIG"][:], scalar1=-1.0, scalar2=None,
                    op0=Alu.mult,
                )

        # ---- final pass + store --------------------------------------
        def stage9(gi):
            g = grp[gi]
            d = FDV[gi]
            dveset = {(i * G) // d + G // (2 * d) for i in range(d)} if d else set()
            a = ACT_V[gi]
            actset = {(i * G) // a + G // (2 * a) for i in range(a)} if a else set()
            for t in range(G):
                ti = gi * G + t
                row0 = ti * P
                v_t = w_pool.tile([P, S], f16, tag="w", name=f"v_{ti}")
                if t in actset:
                    nc.scalar.activation(
                        v_t[:], zs[ti][:], Act.Relu, bias=g["NTAU"][:, t : t + 1],
                    )
                else:
                    nc.vector.tensor_scalar(
                        out=v_t[:], in0=zs[ti][:],
                        scalar1=g["SIG"][:, t : t + 1], scalar2=g["SIG"][:, t : t + 1],
                        op0=Alu.max, op1=Alu.subtract,
                    )
                p_t = p_pool.tile([P, S], f16, tag="p", name=f"p_{ti}")
                if t in dveset:
                    nc.vector.tensor_tensor(
                        out=p_t[:], in0=v_t[:], in1=v_t[:], op=Alu.mult,
                    )
                else:
                    nc.scalar.activation(p_t[:], v_t[:], Act.Square)
                nc.sync.dma_start(out_d[row0 : row0 + P, :], p_t[:])

        def front(gi):
            stage2(gi)
            probe(gi, fcs, WC, r_pool, "rc", 0)
            quad(gi, WC)
            probe(gi, fbs, WB, rb_pool, "rb", 2)
            newton(gi, WB)

        # ---- software-pipelined issue order (NG=2) -------------------
        stage1(0)
        front(0)
        stage9(0)
        stage1(1)
        front(1)
        stage9(1)

    nc.compile()
    return nc


def _get_program():
    if "nc" not in _CACHE:
        _CACHE["nc"] = _build_program()
    return _CACHE["nc"]


def _prep_z16(scores, mask_b):
    z16 = scores.astype(np.float16)
    np.copyto(z16, np.float16(NEG), where=~mask_b)
    return np.ascontiguousarray(z16)


def _kernel_numpy_fallback(scores, mask, alpha):
    """Reference-equivalent host computation (only for alpha != 1.5)."""
    f32 = np.float32
    alpha = max(float(alpha), 1.0)
    am1 = alpha - 1.0
    x = np.where(mask, scores, f32(-1e9)).astype(f32)
    Xs = (x * f32(am1)).astype(f32)
    mx = Xs.max(axis=-1, keepdims=True)
    tau_lo = mx - f32(1.0)
    tau_hi = mx - f32((1.0 / x.shape[-1]) ** am1)
    dm = tau_hi - tau_lo
    tau_m = tau_lo
    inv = f32(1.0 / am1)
    for _ in range(50):
        dm = dm / 2
        tau_m = tau_lo + dm
        p = np.clip(Xs - tau_m, 0.0, None) ** inv
        f = p.sum(axis=-1, keepdims=True) - 1.0
        tau_lo = np.where(f >= 0, tau_m, tau_lo)
    p = np.clip(Xs - tau_m, 0.0, None) ** inv
    return (p / p.sum(axis=-1, keepdims=True)).astype(f32)


def kernel(scores, mask, alpha):
    scores = np.asarray(scores, dtype=np.float32)
    mask_b = np.asarray(mask).astype(bool)
    alpha_v = float(np.asarray(alpha))

    if abs(max(alpha_v, 1.0) - 1.5) > 1e-6:
        return _kernel_numpy_fallback(scores, mask_b, alpha_v)

    z16 = _prep_z16(scores, mask_b)

    from concourse import bass_utils

    nc = _get_program()
    in_maps = [{"z16": z16[i * BP : (i + 1) * BP]} for i in range(N_CORES)]
    res = bass_utils.run_bass_kernel_spmd(nc, in_maps, core_ids=list(range(N_CORES)))
    outs = []
    for r in res.results:
        p = r["out"].astype(np.float32)
        Z = p.sum(axis=1)
        p /= np.maximum(Z, 1e-9)[:, None]
        outs.append(p)
    return np.concatenate(outs, axis=0)
